# revision 4
# baseline (speedup 1.0000x reference)
"""DeepSeek-MoE feed-forward (top-2 of 8 experts) Trainium2 kernel.

Strategy: expert-parallel with host-side routing (the sharding_hint's
"dispatch tokens by topk_idx" option):
  - host computes router logits/softmax/top-2/balanced gates in fp64
    (0.1% of total FLOPs; rankings verified stable vs the fp32 reference),
  - host shards by expert: core e receives expert e's routed tokens packed
    into a dense xT [H, SLOTS] bf16 array (SLOTS = max expert count rounded
    up to 64, so the compiled module is shared by all cores under SPMD),
  - each core runs a pure dense GEMM chain on the PE array in bf16:
    h = silu(w1[e]^T x) with fp32 PSUM, yT = w2[e]^T h, slots always the
    moving (free) dimension in 512-wide blocks so every PSUM tile is one
    full bank; sigmoid on the Act engine, the silu multiply on DVE, PSUM->
    SBUF y copies on Pool,
  - host gathers yT per expert and combines: out[t] = g0*y[t,e0]+g1*y[t,e1]
    (two vectorized gathers; exact fp32 math on the gates).

No collectives and no indirect DMA: the only device work is the 103 GFLOP
of expert MLP compute spread evenly (~12.9 GFLOP + <7% padding per core),
hard against the bf16 PE roofline.

kernel(**inputs) takes the FULL unsharded inputs and returns the FULL output.
"""

import math

import numpy as np
import ml_dtypes

import concourse.bass as bass
import concourse.mybir as mybir
import concourse.tile as tile_mod

P = 128
F32 = mybir.dt.float32
BF16 = mybir.dt.bfloat16
AF = mybir.ActivationFunctionType

N_CORES = 8
DECAY = 0.9
EPS = 0.01
TOP_K = 2


# --------------------------------------------------------------------------
# Workaround for this walrus build: instructions accept only ONE sync wait
# (setupSyncWait "Too many sync wait commands"). Post-process the BIR JSON to
# hoist extra waits onto injected same-engine NoOp carrier instructions, which
# execute in-order on the engine's sequencer right before the instruction.
def _split_multi_waits(raw: bytes) -> bytes:
    import json

    d = json.loads(raw)
    ctr = 0
    changed = False
    for fn in d.get("functions", []):
        for bb in fn.get("blocks", []):
            insts = bb.get("instructions", [])
            out = []
            for inst in insts:
                si = inst.get("sync_info")
                waits = (si.get("on_wait") or []) if si else []
                if len(waits) > 1:
                    changed = True
                    for w in waits[:-1]:
                        nop = {
                            "engine": inst["engine"],
                            "ins": [],
                            "name": f"nopw-{ctr}",
                            "opcode": "NoOp",
                            "outs": [],
                            "sync_info": {"on_update": [], "on_wait": [w]},
                        }
                        if "debug" in inst:
                            nop["debug"] = inst["debug"]
                        ctr += 1
                        out.append(nop)
                    si["on_wait"] = [waits[-1]]
                out.append(inst)
            bb["instructions"] = out
    if not changed:
        return raw
    return json.dumps(d).encode()


def _install_tile_patch():
    if getattr(bass.Bass, "_wait_split_patched", False):
        return
    orig = bass.Bass.to_json_bytes

    def patched(self):
        return _split_multi_waits(orig(self))

    bass.Bass.to_json_bytes = patched
    bass.Bass._wait_split_patched = True


# --------------------------------------------------------------------------
class Cfg:
    def __init__(self, H=768, I=2048, SLOTS=2176, BW=512, n_cores=8):
        assert H % P == 0 and I % P == 0
        self.H, self.I, self.SLOTS, self.BW = H, I, SLOTS, BW
        self.n_cores = n_cores
        self.HC = H // P
        self.IC = I // P
        # slot blocks (ragged last allowed; slots are always a free dim)
        self.blocks = []
        off = 0
        while off < SLOTS:
            bw = min(BW, SLOTS - off)
            self.blocks.append((off, bw))
            off += bw


def build_moe(nc, cfg: Cfg):
    c = cfg
    xT = nc.dram_tensor("xT", [c.HC, P, c.SLOTS], BF16, kind="ExternalInput")
    w1T = nc.dram_tensor("w1T", [c.HC, P, c.I], BF16, kind="ExternalInput")
    w2T = nc.dram_tensor("w2T", [c.IC, P, c.H], BF16, kind="ExternalInput")
    yT = nc.dram_tensor("yT", [c.HC, P, c.SLOTS], BF16, kind="ExternalOutput")

    with tile_mod.TileContext(nc) as tc:
        _emit(tc, cfg, xT, w1T, w2T, yT)
    return nc


def _emit(tc, c: Cfg, xT, w1T, w2T, yT):
    nc = tc.nc
    ctxs = []

    def pool(**kw):
        p = tc.tile_pool(**kw)
        ctxs.append(p)
        return p.__enter__()

    keep = pool(name="keep", bufs=1)
    hp = pool(name="hp", bufs=2 * c.IC)
    yp = pool(name="yp", bufs=2)
    psum = pool(name="psum", bufs=1, space="PSUM")

    # ---- persistent loads ----------------------------------------------
    # x on SP, w1 on Act (only 6 issues before the first sigmoid), w2 on SP
    # after x (first mm2 needs it ~40us in; SP is idle otherwise).
    xt = []
    for kc in range(c.HC):
        t = keep.tile([P, c.SLOTS], BF16, name=f"xt{kc}")
        nc.sync.dma_start(out=t[:], in_=xT[kc])
        xt.append(t)
    w1t = []
    for kc in range(c.HC):
        t = keep.tile([P, c.I], BF16, name=f"w1t{kc}")
        nc.scalar.dma_start(out=t[:], in_=w1T[kc])
        w1t.append(t)
    w2t = []
    for k2 in range(c.IC):
        t = keep.tile([P, c.H], BF16, name=f"w2t{k2}")
        nc.sync.dma_start(out=t[:], in_=w2T[k2])
        w2t.append(t)

    hs = {}

    def mm1(b):
        boff, bw = c.blocks[b]
        for mi in range(c.IC):
            ph = psum.tile([P, c.BW], F32, space="PSUM", name="ph", bufs=3)
            for kc in range(c.HC):
                nc.tensor.matmul(
                    ph[:, :bw],
                    lhsT=w1t[kc][:, mi * P : (mi + 1) * P],
                    rhs=xt[kc][:, boff : boff + bw],
                    start=(kc == 0),
                    stop=(kc == c.HC - 1),
                )
            ht = hp.tile([P, c.BW], BF16, name="ht")
            # silu(x) = x * sigmoid(x); Act computes the sigmoid, DVE the mul
            nc.scalar.activation(ht[:, :bw], ph[:, :bw], AF.Sigmoid)
            nc.vector.tensor_mul(ht[:, :bw], ht[:, :bw], ph[:, :bw])
            hs[(b, mi)] = ht

    def mm2(b):
        boff, bw = c.blocks[b]
        yt = yp.tile([P, c.HC, c.BW], BF16, name="yt")
        for hn in range(c.HC):
            py = psum.tile([P, c.BW], F32, space="PSUM", name="py", bufs=3)
            for k2 in range(c.IC):
                nc.tensor.matmul(
                    py[:, :bw],
                    lhsT=w2t[k2][:, hn * P : (hn + 1) * P],
                    rhs=hs[(b, k2)][:, :bw],
                    start=(k2 == 0),
                    stop=(k2 == c.IC - 1),
                )
            # Pool/GPSIMD cannot read PSUM on HW; split the drain copies
            # between DVE and Act
            if hn % 2 == 0:
                nc.vector.tensor_copy(out=yt[:, hn, :bw], in_=py[:, :bw])
            else:
                nc.scalar.activation(yt[:, hn, :bw], py[:, :bw], AF.Copy)
            nc.sync.dma_start(out=yT[hn][:, boff : boff + bw], in_=yt[:, hn, :bw])
        for mi in range(c.IC):
            del hs[(b, mi)]

    # 1-block skew keeps the PE stream dense across the mm1->mm2 boundary
    nb = len(c.blocks)
    mm1(0)
    for b in range(nb):
        if b + 1 < nb:
            mm1(b + 1)
        mm2(b)

    for p in reversed(ctxs):
        p.__exit__(None, None, None)


# --------------------------------------------------------------------------
def route_host(flat, router_w):
    """fp64 router: logits, softmax, top-2, load-balanced gates.

    Returns (top2 [T,2], gates [T,2] fp64, perm [2T] pair order sorted by
    expert, counts [E])."""
    lg = flat.astype(np.float64) @ router_w.astype(np.float64).T
    order = np.argsort(-lg, axis=1, kind="stable")
    top2 = order[:, :TOP_K]
    mx = lg.max(axis=1, keepdims=True)
    ex = np.exp(lg - mx)
    probs = ex / ex.sum(axis=1, keepdims=True)
    topk_probs = np.take_along_axis(probs, top2, axis=1)
    imp = probs.sum(axis=0)
    running = 1.0 + (1.0 - DECAY) * (imp - 1.0) + EPS
    bal = topk_probs / running[top2]
    gates = bal / bal.sum(axis=1, keepdims=True)
    keys = top2.ravel()
    perm = np.argsort(keys, kind="stable")
    counts = np.bincount(keys, minlength=router_w.shape[0])
    return top2, gates, perm, counts


def host_prep(flat, router_w, w1, w2, cfg: Cfg):
    """Shard by expert: pack each expert's routed tokens (pair order) into a
    dense transposed bf16 array per core."""
    c = cfg
    bf16 = ml_dtypes.bfloat16
    E = router_w.shape[0]
    assert E == c.n_cores
    top2, gates, perm, counts = route_host(flat, router_w)
    assert counts.max() <= c.SLOTS, (counts.max(), c.SLOTS)
    tok = perm // TOP_K
    starts = np.concatenate([[0], np.cumsum(counts)])
    w1T = np.ascontiguousarray(w1.transpose(0, 2, 1)).astype(bf16)  # [E, H, I]
    w2T = np.ascontiguousarray(w2.transpose(0, 2, 1)).astype(bf16)  # [E, I, H]
    in_maps = []
    for e in range(E):
        sel = tok[starts[e] : starts[e + 1]]
        xTe = np.zeros((c.H, c.SLOTS), dtype=bf16)
        xTe[:, : len(sel)] = flat[sel].astype(bf16).T
        in_maps.append({
            "xT": xTe.reshape(c.HC, P, c.SLOTS),
            "w1T": w1T[e].reshape(c.HC, P, c.I),
            "w2T": w2T[e].reshape(c.IC, P, c.H),
        })
    return in_maps, (gates, perm, counts)


def host_combine(outs, gates, perm, counts, cfg: Cfg):
    """outs[e] = yT [HC, P, SLOTS] bf16 -> out[t] = sum_k g_k * y[t, e_k]."""
    c = cfg
    T = gates.shape[0]
    ys = [
        np.asarray(outs[e]).reshape(c.H, c.SLOTS).T[: counts[e]] for e in range(len(outs))
    ]
    y_sorted = np.concatenate(ys, axis=0).astype(np.float32)  # [2T, H] pair-sorted
    y_pair = np.empty_like(y_sorted)
    y_pair[perm] = y_sorted
    g = gates.astype(np.float32)
    return y_pair[0::2] * g[:, :1] + y_pair[1::2] * g[:, 1:2]


_CACHED = {}


def _get_nc(cfg: Cfg):
    key = (cfg.H, cfg.I, cfg.SLOTS, cfg.BW, cfg.n_cores)
    if key not in _CACHED:
        _install_tile_patch()
        nc = bass.Bass("TRN2", num_devices=cfg.n_cores)
        build_moe(nc, cfg)
        _CACHED[key] = nc
    return _CACHED[key]


def pick_slots(counts):
    return max(int(-(-counts.max() // 64)) * 64, P)


def run(hidden_states, router_w, w1, w2, cfg: Cfg = None, **run_kwargs):
    from concourse.bass_utils import run_bass_kernel_spmd

    B, S, H = hidden_states.shape
    flat = np.ascontiguousarray(hidden_states.reshape(-1, H).astype(np.float32))
    if cfg is None:
        _, _, _, counts = route_host(flat, router_w)
        cfg = Cfg(H=H, I=w1.shape[1], SLOTS=pick_slots(counts),
                  n_cores=router_w.shape[0])
    nc = _get_nc(cfg)
    in_maps, (gates, perm, counts) = host_prep(flat, router_w, w1, w2, cfg)
    res = run_bass_kernel_spmd(
        nc, in_maps, core_ids=list(range(cfg.n_cores)), **run_kwargs
    )
    outs = [res.results[i]["yT"] for i in range(cfg.n_cores)]
    full = host_combine(outs, gates, perm, counts, cfg)
    return full, res


def kernel(hidden_states, router_w, w1, w2):
    hidden_states = np.asarray(hidden_states, dtype=np.float32)
    router_w = np.asarray(router_w, dtype=np.float32)
    w1 = np.asarray(w1, dtype=np.float32)
    w2 = np.asarray(w2, dtype=np.float32)
    B, S, H = hidden_states.shape
    full, _ = run(hidden_states, router_w, w1, w2)
    return full.reshape(B, S, H).astype(np.float32)


# revision 7
# speedup vs baseline: 1.0139x; 1.0139x over previous
"""DeepSeek-MoE feed-forward (top-2 of 8 experts) Trainium2 kernel.

Strategy: expert-parallel with host-side routing (the sharding_hint's
"dispatch tokens by topk_idx" option):
  - host computes router logits/softmax/top-2/balanced gates in fp64
    (0.1% of total FLOPs; rankings verified stable vs the fp32 reference),
  - host shards by expert: core e receives expert e's routed tokens packed
    into a dense xT [H, SLOTS] bf16 array (SLOTS = max expert count rounded
    up to 64, so the compiled module is shared by all cores under SPMD),
  - each core runs a pure dense GEMM chain on the PE array in bf16:
    h = silu(w1[e]^T x) with fp32 PSUM, yT = w2[e]^T h, slots always the
    moving (free) dimension in 512-wide blocks so every PSUM tile is one
    full bank; sigmoid on the Act engine, the silu multiply on DVE, PSUM->
    SBUF y copies on Pool,
  - host gathers yT per expert and combines: out[t] = g0*y[t,e0]+g1*y[t,e1]
    (two vectorized gathers; exact fp32 math on the gates).

No collectives and no indirect DMA: the only device work is the 103 GFLOP
of expert MLP compute spread evenly (~12.9 GFLOP + <7% padding per core),
hard against the bf16 PE roofline.

kernel(**inputs) takes the FULL unsharded inputs and returns the FULL output.
"""

import math

import numpy as np
import ml_dtypes

import concourse.bass as bass
import concourse.mybir as mybir
import concourse.tile as tile_mod

P = 128
F32 = mybir.dt.float32
BF16 = mybir.dt.bfloat16
AF = mybir.ActivationFunctionType

N_CORES = 8
DECAY = 0.9
EPS = 0.01
TOP_K = 2


# --------------------------------------------------------------------------
# Workaround for this walrus build: instructions accept only ONE sync wait
# (setupSyncWait "Too many sync wait commands"). Post-process the BIR JSON to
# hoist extra waits onto injected same-engine NoOp carrier instructions, which
# execute in-order on the engine's sequencer right before the instruction.
def _split_multi_waits(raw: bytes) -> bytes:
    import json

    d = json.loads(raw)
    ctr = 0
    changed = False
    for fn in d.get("functions", []):
        for bb in fn.get("blocks", []):
            insts = bb.get("instructions", [])
            out = []
            for inst in insts:
                si = inst.get("sync_info")
                waits = (si.get("on_wait") or []) if si else []
                if len(waits) > 1:
                    changed = True
                    for w in waits[:-1]:
                        nop = {
                            "engine": inst["engine"],
                            "ins": [],
                            "name": f"nopw-{ctr}",
                            "opcode": "NoOp",
                            "outs": [],
                            "sync_info": {"on_update": [], "on_wait": [w]},
                        }
                        if "debug" in inst:
                            nop["debug"] = inst["debug"]
                        ctr += 1
                        out.append(nop)
                    si["on_wait"] = [waits[-1]]
                out.append(inst)
            bb["instructions"] = out
    if not changed:
        return raw
    return json.dumps(d).encode()


def _install_tile_patch():
    if getattr(bass.Bass, "_wait_split_patched", False):
        return
    orig = bass.Bass.to_json_bytes

    def patched(self):
        return _split_multi_waits(orig(self))

    bass.Bass.to_json_bytes = patched
    bass.Bass._wait_split_patched = True


# --------------------------------------------------------------------------
class Cfg:
    def __init__(self, H=768, I=2048, SLOTS=2176, BW=512, n_cores=8):
        assert H % P == 0 and I % P == 0
        self.H, self.I, self.SLOTS, self.BW = H, I, SLOTS, BW
        self.n_cores = n_cores
        self.HC = H // P
        self.IC = I // P
        # slot blocks (ragged last allowed; slots are always a free dim)
        self.blocks = []
        off = 0
        while off < SLOTS:
            bw = min(BW, SLOTS - off)
            self.blocks.append((off, bw))
            off += bw


def build_moe(nc, cfg: Cfg):
    c = cfg
    xT = nc.dram_tensor("xT", [c.HC, P, c.SLOTS], BF16, kind="ExternalInput")
    # w1 is stored mi-major ([IC, P, HC*128], host-pretransposed) so one DMA
    # delivers exactly the lhsT columns of one mm1 PSUM group across all kc
    w1T = nc.dram_tensor("w1T", [c.IC, P, c.HC * P], BF16, kind="ExternalInput")
    w2T = nc.dram_tensor("w2T", [c.IC, P, c.H], BF16, kind="ExternalInput")
    yT = nc.dram_tensor("yT", [c.HC, P, c.SLOTS], BF16, kind="ExternalOutput")

    with tile_mod.TileContext(nc) as tc:
        _emit(tc, cfg, xT, w1T, w2T, yT)
    return nc


def _emit(tc, c: Cfg, xT, w1T, w2T, yT):
    nc = tc.nc
    ctxs = []

    def pool(**kw):
        p = tc.tile_pool(**kw)
        ctxs.append(p)
        return p.__enter__()

    keep = pool(name="keep", bufs=1)
    hp = pool(name="hp", bufs=2 * c.IC)
    yp = pool(name="yp", bufs=2)
    psum = pool(name="psum", bufs=1, space="PSUM")

    # ---- persistent tiles ----------------------------------------------
    # All input DMAs are issued on SP in the order PE consumes the data:
    # x block 0 (6 small column-slices), then the 16 mi-major w1 chunks,
    # then the remaining x blocks. w2 goes on the otherwise-idle Pool
    # engine (first needed ~40us in). y writeback also on Pool.
    xt = [keep.tile([P, c.SLOTS], BF16, name=f"xt{kc}") for kc in range(c.HC)]

    def load_x_block(b):
        boff, bw = c.blocks[b]
        for kc in range(c.HC):
            nc.sync.dma_start(
                out=xt[kc][:, boff : boff + bw], in_=xT[kc][:, boff : boff + bw]
            )

    load_x_block(0)
    w1m = []
    for mi in range(c.IC):
        t = keep.tile([P, c.HC, P], BF16, name=f"w1m{mi}")
        nc.sync.dma_start(out=t[:], in_=w1T[mi])
        w1m.append(t)
    for b in range(1, len(c.blocks)):
        load_x_block(b)
    w2t = []
    for k2 in range(c.IC):
        t = keep.tile([P, c.H], BF16, name=f"w2t{k2}")
        nc.gpsimd.dma_start(out=t[:], in_=w2T[k2])
        w2t.append(t)

    hs = {}

    def mm1(b):
        boff, bw = c.blocks[b]
        for mi in range(c.IC):
            ph = psum.tile([P, c.BW], F32, space="PSUM", name="ph", bufs=3)
            for kc in range(c.HC):
                nc.tensor.matmul(
                    ph[:, :bw],
                    lhsT=w1m[mi][:, kc, :],
                    rhs=xt[kc][:, boff : boff + bw],
                    start=(kc == 0),
                    stop=(kc == c.HC - 1),
                )
            ht = hp.tile([P, c.BW], BF16, name="ht")
            # silu(x) = x * sigmoid(x); Act computes the sigmoid, DVE the mul
            nc.scalar.activation(ht[:, :bw], ph[:, :bw], AF.Sigmoid)
            nc.vector.tensor_mul(ht[:, :bw], ht[:, :bw], ph[:, :bw])
            hs[(b, mi)] = ht

    def mm2(b):
        boff, bw = c.blocks[b]
        yt = yp.tile([P, c.HC, c.BW], BF16, name="yt")
        for hn in range(c.HC):
            py = psum.tile([P, c.BW], F32, space="PSUM", name="py", bufs=3)
            for k2 in range(c.IC):
                nc.tensor.matmul(
                    py[:, :bw],
                    lhsT=w2t[k2][:, hn * P : (hn + 1) * P],
                    rhs=hs[(b, k2)][:, :bw],
                    start=(k2 == 0),
                    stop=(k2 == c.IC - 1),
                )
            # Pool/GPSIMD cannot read PSUM on HW; split the drain copies
            # between DVE and Act
            if hn % 2 == 0:
                nc.vector.tensor_copy(out=yt[:, hn, :bw], in_=py[:, :bw])
            else:
                nc.scalar.activation(yt[:, hn, :bw], py[:, :bw], AF.Copy)
            nc.gpsimd.dma_start(out=yT[hn][:, boff : boff + bw], in_=yt[:, hn, :bw])
        for mi in range(c.IC):
            del hs[(b, mi)]

    # 1-block skew keeps the PE stream dense across the mm1->mm2 boundary
    nb = len(c.blocks)
    mm1(0)
    for b in range(nb):
        if b + 1 < nb:
            mm1(b + 1)
        mm2(b)

    for p in reversed(ctxs):
        p.__exit__(None, None, None)


# --------------------------------------------------------------------------
def route_host(flat, router_w):
    """fp64 router: logits, softmax, top-2, load-balanced gates.

    Returns (top2 [T,2], gates [T,2] fp64, perm [2T] pair order sorted by
    expert, counts [E])."""
    lg = flat.astype(np.float64) @ router_w.astype(np.float64).T
    order = np.argsort(-lg, axis=1, kind="stable")
    top2 = order[:, :TOP_K]
    mx = lg.max(axis=1, keepdims=True)
    ex = np.exp(lg - mx)
    probs = ex / ex.sum(axis=1, keepdims=True)
    topk_probs = np.take_along_axis(probs, top2, axis=1)
    imp = probs.sum(axis=0)
    running = 1.0 + (1.0 - DECAY) * (imp - 1.0) + EPS
    bal = topk_probs / running[top2]
    gates = bal / bal.sum(axis=1, keepdims=True)
    keys = top2.ravel()
    perm = np.argsort(keys, kind="stable")
    counts = np.bincount(keys, minlength=router_w.shape[0])
    return top2, gates, perm, counts


def host_prep(flat, router_w, w1, w2, cfg: Cfg):
    """Shard by expert: pack each expert's routed tokens (pair order) into a
    dense transposed bf16 array per core."""
    c = cfg
    bf16 = ml_dtypes.bfloat16
    E = router_w.shape[0]
    assert E == c.n_cores
    top2, gates, perm, counts = route_host(flat, router_w)
    assert counts.max() <= c.SLOTS, (counts.max(), c.SLOTS)
    tok = perm // TOP_K
    starts = np.concatenate([[0], np.cumsum(counts)])
    # w1 mi-major: W[mi, p, kc*128+j] = w1[e][mi*128+j, kc*128+p]
    w1m = np.ascontiguousarray(
        w1.reshape(E, c.IC, P, c.HC, P).transpose(0, 1, 4, 3, 2)
        .reshape(E, c.IC, P, c.HC * P)
    ).astype(bf16)
    w2T = np.ascontiguousarray(w2.transpose(0, 2, 1)).astype(bf16)  # [E, I, H]
    in_maps = []
    for e in range(E):
        sel = tok[starts[e] : starts[e + 1]]
        xTe = np.zeros((c.H, c.SLOTS), dtype=bf16)
        xTe[:, : len(sel)] = flat[sel].astype(bf16).T
        in_maps.append({
            "xT": xTe.reshape(c.HC, P, c.SLOTS),
            "w1T": w1m[e],
            "w2T": w2T[e].reshape(c.IC, P, c.H),
        })
    return in_maps, (gates, perm, counts)


def host_combine(outs, gates, perm, counts, cfg: Cfg):
    """outs[e] = yT [HC, P, SLOTS] bf16 -> out[t] = sum_k g_k * y[t, e_k]."""
    c = cfg
    T = gates.shape[0]
    ys = [
        np.asarray(outs[e]).reshape(c.H, c.SLOTS).T[: counts[e]] for e in range(len(outs))
    ]
    y_sorted = np.concatenate(ys, axis=0).astype(np.float32)  # [2T, H] pair-sorted
    y_pair = np.empty_like(y_sorted)
    y_pair[perm] = y_sorted
    g = gates.astype(np.float32)
    return y_pair[0::2] * g[:, :1] + y_pair[1::2] * g[:, 1:2]


_CACHED = {}


def _get_nc(cfg: Cfg):
    key = (cfg.H, cfg.I, cfg.SLOTS, cfg.BW, cfg.n_cores)
    if key not in _CACHED:
        _install_tile_patch()
        nc = bass.Bass("TRN2", num_devices=cfg.n_cores)
        build_moe(nc, cfg)
        _CACHED[key] = nc
    return _CACHED[key]


def pick_slots(counts):
    return max(int(-(-counts.max() // 64)) * 64, P)


def run(hidden_states, router_w, w1, w2, cfg: Cfg = None, **run_kwargs):
    from concourse.bass_utils import run_bass_kernel_spmd

    B, S, H = hidden_states.shape
    flat = np.ascontiguousarray(hidden_states.reshape(-1, H).astype(np.float32))
    if cfg is None:
        _, _, _, counts = route_host(flat, router_w)
        cfg = Cfg(H=H, I=w1.shape[1], SLOTS=pick_slots(counts),
                  n_cores=router_w.shape[0])
    nc = _get_nc(cfg)
    in_maps, (gates, perm, counts) = host_prep(flat, router_w, w1, w2, cfg)
    res = run_bass_kernel_spmd(
        nc, in_maps, core_ids=list(range(cfg.n_cores)), **run_kwargs
    )
    outs = [res.results[i]["yT"] for i in range(cfg.n_cores)]
    full = host_combine(outs, gates, perm, counts, cfg)
    return full, res


def kernel(hidden_states, router_w, w1, w2):
    hidden_states = np.asarray(hidden_states, dtype=np.float32)
    router_w = np.asarray(router_w, dtype=np.float32)
    w1 = np.asarray(w1, dtype=np.float32)
    w2 = np.asarray(w2, dtype=np.float32)
    B, S, H = hidden_states.shape
    full, _ = run(hidden_states, router_w, w1, w2)
    return full.reshape(B, S, H).astype(np.float32)


# revision 9
# speedup vs baseline: 1.0591x; 1.0446x over previous
"""DeepSeek-MoE feed-forward (top-2 of 8 experts) Trainium2 kernel.

Strategy: expert-parallel with host-side routing (the sharding_hint's
"dispatch tokens by topk_idx" option):
  - host computes router logits/softmax/top-2/balanced gates in fp64
    (0.1% of total FLOPs; rankings verified stable vs the fp32 reference),
  - host shards by expert: core e receives expert e's routed tokens packed
    into a dense xT [H, SLOTS] bf16 array (SLOTS = max expert count rounded
    up to 64, so the compiled module is shared by all cores under SPMD),
  - each core runs a pure dense GEMM chain on the PE array in bf16:
    h = silu(w1[e]^T x) with fp32 PSUM, yT = w2[e]^T h, slots always the
    moving (free) dimension in 512-wide blocks so every PSUM tile is one
    full bank; sigmoid on the Act engine, the silu multiply on DVE, PSUM->
    SBUF y copies on Pool,
  - host gathers yT per expert and combines: out[t] = g0*y[t,e0]+g1*y[t,e1]
    (two vectorized gathers; exact fp32 math on the gates).

No collectives and no indirect DMA: the only device work is the 103 GFLOP
of expert MLP compute spread evenly (~12.9 GFLOP + <7% padding per core),
hard against the bf16 PE roofline.

kernel(**inputs) takes the FULL unsharded inputs and returns the FULL output.
"""

import math

import numpy as np
import ml_dtypes

import concourse.bass as bass
import concourse.mybir as mybir
import concourse.tile as tile_mod

P = 128
F32 = mybir.dt.float32
BF16 = mybir.dt.bfloat16
AF = mybir.ActivationFunctionType

N_CORES = 8
DECAY = 0.9
EPS = 0.01
TOP_K = 2


# --------------------------------------------------------------------------
# Workaround for this walrus build: instructions accept only ONE sync wait
# (setupSyncWait "Too many sync wait commands"). Post-process the BIR JSON to
# hoist extra waits onto injected same-engine NoOp carrier instructions, which
# execute in-order on the engine's sequencer right before the instruction.
def _split_multi_waits(raw: bytes) -> bytes:
    import json

    d = json.loads(raw)
    ctr = 0
    changed = False
    for fn in d.get("functions", []):
        for bb in fn.get("blocks", []):
            insts = bb.get("instructions", [])
            out = []
            for inst in insts:
                si = inst.get("sync_info")
                waits = (si.get("on_wait") or []) if si else []
                if len(waits) > 1:
                    changed = True
                    for w in waits[:-1]:
                        nop = {
                            "engine": inst["engine"],
                            "ins": [],
                            "name": f"nopw-{ctr}",
                            "opcode": "NoOp",
                            "outs": [],
                            "sync_info": {"on_update": [], "on_wait": [w]},
                        }
                        if "debug" in inst:
                            nop["debug"] = inst["debug"]
                        ctr += 1
                        out.append(nop)
                    si["on_wait"] = [waits[-1]]
                out.append(inst)
            bb["instructions"] = out
    if not changed:
        return raw
    return json.dumps(d).encode()


def _install_tile_patch():
    if getattr(bass.Bass, "_wait_split_patched", False):
        return
    orig = bass.Bass.to_json_bytes

    def patched(self):
        return _split_multi_waits(orig(self))

    bass.Bass.to_json_bytes = patched
    bass.Bass._wait_split_patched = True


# --------------------------------------------------------------------------
class Cfg:
    def __init__(self, H=768, I=2048, SLOTS=2176, BW=512, n_cores=8):
        assert H % P == 0 and I % P == 0
        self.H, self.I, self.SLOTS, self.BW = H, I, SLOTS, BW
        self.n_cores = n_cores
        self.HC = H // P
        self.IC = I // P
        # slot blocks (ragged last allowed; slots are always a free dim)
        self.blocks = []
        off = 0
        while off < SLOTS:
            bw = min(BW, SLOTS - off)
            self.blocks.append((off, bw))
            off += bw


def build_moe(nc, cfg: Cfg):
    c = cfg
    xT = nc.dram_tensor("xT", [c.HC, P, c.SLOTS], BF16, kind="ExternalInput")
    # w1 is stored mi-major ([IC, P, HC*128], host-pretransposed) so one DMA
    # delivers exactly the lhsT columns of one mm1 PSUM group across all kc
    w1T = nc.dram_tensor("w1T", [c.IC, P, c.HC * P], BF16, kind="ExternalInput")
    w2T = nc.dram_tensor("w2T", [c.IC, P, c.H], BF16, kind="ExternalInput")
    yT = nc.dram_tensor("yT", [c.HC, P, c.SLOTS], BF16, kind="ExternalOutput")

    with tile_mod.TileContext(nc) as tc:
        _emit(tc, cfg, xT, w1T, w2T, yT)
    return nc


def _emit(tc, c: Cfg, xT, w1T, w2T, yT):
    nc = tc.nc
    ctxs = []

    def pool(**kw):
        p = tc.tile_pool(**kw)
        ctxs.append(p)
        return p.__enter__()

    keep = pool(name="keep", bufs=1)
    hp = pool(name="hp", bufs=2 * c.IC)
    yp = pool(name="yp", bufs=2)
    psum = pool(name="psum", bufs=1, space="PSUM")

    # ---- persistent tiles ----------------------------------------------
    # All input DMAs are issued on SP in the order PE consumes the data:
    # x block 0 (6 small column-slices), then the 16 mi-major w1 chunks,
    # then the remaining x blocks. w2 goes on the otherwise-idle Pool
    # engine (first needed ~40us in). y writeback also on Pool.
    xt = [keep.tile([P, c.SLOTS], BF16, name=f"xt{kc}") for kc in range(c.HC)]

    def load_x_block(b):
        boff, bw = c.blocks[b]
        for kc in range(c.HC):
            nc.sync.dma_start(
                out=xt[kc][:, boff : boff + bw], in_=xT[kc][:, boff : boff + bw]
            )

    # first few w1 chunks race ahead on the Act queue (its sigmoids start
    # later), unblocking the first PSUM groups while x block 0 streams on SP
    w1m = []
    for mi in range(c.IC):
        t = keep.tile([P, c.HC, P], BF16, name=f"w1m{mi}")
        if mi < 3:
            nc.scalar.dma_start(out=t[:], in_=w1T[mi])
        w1m.append(t)
    load_x_block(0)
    for mi in range(3, c.IC):
        nc.sync.dma_start(out=w1m[mi][:], in_=w1T[mi])
    for b in range(1, len(c.blocks)):
        load_x_block(b)
    w2t = []
    for k2 in range(c.IC):
        t = keep.tile([P, c.H], BF16, name=f"w2t{k2}")
        nc.gpsimd.dma_start(out=t[:], in_=w2T[k2])
        w2t.append(t)

    hs = {}

    def mm1(b):
        boff, bw = c.blocks[b]
        for mi in range(c.IC):
            ph = psum.tile([P, c.BW], F32, space="PSUM", name="ph", bufs=3)
            for kc in range(c.HC):
                nc.tensor.matmul(
                    ph[:, :bw],
                    lhsT=w1m[mi][:, kc, :],
                    rhs=xt[kc][:, boff : boff + bw],
                    start=(kc == 0),
                    stop=(kc == c.HC - 1),
                )
            ht = hp.tile([P, c.BW], BF16, name="ht")
            # silu(x) = x * sigmoid(x); Act computes the sigmoid, DVE the mul
            nc.scalar.activation(ht[:, :bw], ph[:, :bw], AF.Sigmoid)
            nc.vector.tensor_mul(ht[:, :bw], ht[:, :bw], ph[:, :bw])
            hs[(b, mi)] = ht

    def mm2(b):
        boff, bw = c.blocks[b]
        yt = yp.tile([P, c.HC, c.BW], BF16, name="yt")
        for hn in range(c.HC):
            py = psum.tile([P, c.BW], F32, space="PSUM", name="py", bufs=3)
            for k2 in range(c.IC):
                nc.tensor.matmul(
                    py[:, :bw],
                    lhsT=w2t[k2][:, hn * P : (hn + 1) * P],
                    rhs=hs[(b, k2)][:, :bw],
                    start=(k2 == 0),
                    stop=(k2 == c.IC - 1),
                )
            # Pool/GPSIMD cannot read PSUM on HW; split the drain copies
            # between DVE and Act
            if hn % 2 == 0:
                nc.vector.tensor_copy(out=yt[:, hn, :bw], in_=py[:, :bw])
            else:
                nc.scalar.activation(yt[:, hn, :bw], py[:, :bw], AF.Copy)
            nc.sync.dma_start(out=yT[hn][:, boff : boff + bw], in_=yt[:, hn, :bw])
        for mi in range(c.IC):
            del hs[(b, mi)]

    # 1-block skew keeps the PE stream dense across the mm1->mm2 boundary
    nb = len(c.blocks)
    mm1(0)
    for b in range(nb):
        if b + 1 < nb:
            mm1(b + 1)
        mm2(b)

    for p in reversed(ctxs):
        p.__exit__(None, None, None)


# --------------------------------------------------------------------------
def route_host(flat, router_w):
    """fp64 router: logits, softmax, top-2, load-balanced gates.

    Returns (top2 [T,2], gates [T,2] fp64, perm [2T] pair order sorted by
    expert, counts [E])."""
    lg = flat.astype(np.float64) @ router_w.astype(np.float64).T
    order = np.argsort(-lg, axis=1, kind="stable")
    top2 = order[:, :TOP_K]
    mx = lg.max(axis=1, keepdims=True)
    ex = np.exp(lg - mx)
    probs = ex / ex.sum(axis=1, keepdims=True)
    topk_probs = np.take_along_axis(probs, top2, axis=1)
    imp = probs.sum(axis=0)
    running = 1.0 + (1.0 - DECAY) * (imp - 1.0) + EPS
    bal = topk_probs / running[top2]
    gates = bal / bal.sum(axis=1, keepdims=True)
    keys = top2.ravel()
    perm = np.argsort(keys, kind="stable")
    counts = np.bincount(keys, minlength=router_w.shape[0])
    return top2, gates, perm, counts


def host_prep(flat, router_w, w1, w2, cfg: Cfg):
    """Shard by expert: pack each expert's routed tokens (pair order) into a
    dense transposed bf16 array per core."""
    c = cfg
    bf16 = ml_dtypes.bfloat16
    E = router_w.shape[0]
    assert E == c.n_cores
    top2, gates, perm, counts = route_host(flat, router_w)
    assert counts.max() <= c.SLOTS, (counts.max(), c.SLOTS)
    tok = perm // TOP_K
    starts = np.concatenate([[0], np.cumsum(counts)])
    # w1 mi-major: W[mi, p, kc*128+j] = w1[e][mi*128+j, kc*128+p]
    w1m = np.ascontiguousarray(
        w1.reshape(E, c.IC, P, c.HC, P).transpose(0, 1, 4, 3, 2)
        .reshape(E, c.IC, P, c.HC * P)
    ).astype(bf16)
    w2T = np.ascontiguousarray(w2.transpose(0, 2, 1)).astype(bf16)  # [E, I, H]
    in_maps = []
    for e in range(E):
        sel = tok[starts[e] : starts[e + 1]]
        xTe = np.zeros((c.H, c.SLOTS), dtype=bf16)
        xTe[:, : len(sel)] = flat[sel].astype(bf16).T
        in_maps.append({
            "xT": xTe.reshape(c.HC, P, c.SLOTS),
            "w1T": w1m[e],
            "w2T": w2T[e].reshape(c.IC, P, c.H),
        })
    return in_maps, (gates, perm, counts)


def host_combine(outs, gates, perm, counts, cfg: Cfg):
    """outs[e] = yT [HC, P, SLOTS] bf16 -> out[t] = sum_k g_k * y[t, e_k]."""
    c = cfg
    T = gates.shape[0]
    ys = [
        np.asarray(outs[e]).reshape(c.H, c.SLOTS).T[: counts[e]] for e in range(len(outs))
    ]
    y_sorted = np.concatenate(ys, axis=0).astype(np.float32)  # [2T, H] pair-sorted
    y_pair = np.empty_like(y_sorted)
    y_pair[perm] = y_sorted
    g = gates.astype(np.float32)
    return y_pair[0::2] * g[:, :1] + y_pair[1::2] * g[:, 1:2]


_CACHED = {}


def _get_nc(cfg: Cfg):
    key = (cfg.H, cfg.I, cfg.SLOTS, cfg.BW, cfg.n_cores)
    if key not in _CACHED:
        _install_tile_patch()
        nc = bass.Bass("TRN2", num_devices=cfg.n_cores)
        build_moe(nc, cfg)
        _CACHED[key] = nc
    return _CACHED[key]


def pick_slots(counts):
    return max(int(-(-counts.max() // 64)) * 64, P)


def run(hidden_states, router_w, w1, w2, cfg: Cfg = None, **run_kwargs):
    from concourse.bass_utils import run_bass_kernel_spmd

    B, S, H = hidden_states.shape
    flat = np.ascontiguousarray(hidden_states.reshape(-1, H).astype(np.float32))
    if cfg is None:
        _, _, _, counts = route_host(flat, router_w)
        cfg = Cfg(H=H, I=w1.shape[1], SLOTS=pick_slots(counts),
                  n_cores=router_w.shape[0])
    nc = _get_nc(cfg)
    in_maps, (gates, perm, counts) = host_prep(flat, router_w, w1, w2, cfg)
    res = run_bass_kernel_spmd(
        nc, in_maps, core_ids=list(range(cfg.n_cores)), **run_kwargs
    )
    outs = [res.results[i]["yT"] for i in range(cfg.n_cores)]
    full = host_combine(outs, gates, perm, counts, cfg)
    return full, res


def kernel(hidden_states, router_w, w1, w2):
    hidden_states = np.asarray(hidden_states, dtype=np.float32)
    router_w = np.asarray(router_w, dtype=np.float32)
    w1 = np.asarray(w1, dtype=np.float32)
    w2 = np.asarray(w2, dtype=np.float32)
    B, S, H = hidden_states.shape
    full, _ = run(hidden_states, router_w, w1, w2)
    return full.reshape(B, S, H).astype(np.float32)


# revision 12
# speedup vs baseline: 1.0954x; 1.0342x over previous
"""DeepSeek-MoE feed-forward (top-2 of 8 experts) Trainium2 kernel.

Strategy: expert-parallel with host-side routing (the sharding_hint's
"dispatch tokens by topk_idx" option):
  - host computes router logits/softmax/top-2/balanced gates in fp64
    (0.1% of total FLOPs; rankings verified stable vs the fp32 reference),
  - host shards by expert into per-core "bins": every core runs the same
    SPMD program over SLOTS = A + B token slots, where columns [0,A) hold
    (up to A) tokens of one expert and columns [A,A+B) tokens of a second
    host-assigned expert.  Bin sizes (A, B) are solved per routing so the
    16 bins cover all expert token counts with minimal padding (for the
    benchmark routing: A=1965, B=128 -> 2093 slots vs a 2146 max count),
  - each core runs a dense GEMM chain on the PE array in bf16: h =
    silu(w1^T x) with fp32 PSUM, yT = w2^T h, slots always the moving
    (free) dimension in <=512-wide blocks (each block single-expert, so
    every PSUM tile is one accumulation chain); sigmoid on Act, the silu
    multiply on DVE, PSUM->SBUF y copies split Act/DVE, input streaming
    ordered so the PE never waits (w1 stored mi-major so one DMA feeds
    exactly one PSUM group),
  - host gathers yT per bin and combines out[t] = g0*y[t,e0] + g1*y[t,e1]
    (vectorized gathers, exact fp32 gates).

No collectives and no indirect DMA: the only device work is the 103 GFLOP
of expert MLP compute spread evenly (~12.9 GFLOP + ~2% padding per core),
hard against the bf16 PE roofline.

kernel(**inputs) takes the FULL unsharded inputs and returns the FULL output.
"""

import numpy as np
import ml_dtypes

import concourse.bass as bass
import concourse.mybir as mybir
import concourse.tile as tile_mod

P = 128
F32 = mybir.dt.float32
BF16 = mybir.dt.bfloat16
AF = mybir.ActivationFunctionType

N_CORES = 8
DECAY = 0.9
EPS = 0.01
TOP_K = 2


# --------------------------------------------------------------------------
# Workaround for this walrus build: instructions accept only ONE sync wait
# (setupSyncWait "Too many sync wait commands"). Post-process the BIR JSON to
# hoist extra waits onto injected same-engine NoOp carrier instructions, which
# execute in-order on the engine's sequencer right before the instruction.
def _split_multi_waits(raw: bytes) -> bytes:
    import json

    d = json.loads(raw)
    ctr = 0
    changed = False
    for fn in d.get("functions", []):
        for bb in fn.get("blocks", []):
            insts = bb.get("instructions", [])
            out = []
            for inst in insts:
                si = inst.get("sync_info")
                waits = (si.get("on_wait") or []) if si else []
                if len(waits) > 1:
                    changed = True
                    for w in waits[:-1]:
                        nop = {
                            "engine": inst["engine"],
                            "ins": [],
                            "name": f"nopw-{ctr}",
                            "opcode": "NoOp",
                            "outs": [],
                            "sync_info": {"on_update": [], "on_wait": [w]},
                        }
                        if "debug" in inst:
                            nop["debug"] = inst["debug"]
                        ctr += 1
                        out.append(nop)
                    si["on_wait"] = [waits[-1]]
                out.append(inst)
            bb["instructions"] = out
    if not changed:
        return raw
    return json.dumps(d).encode()


def _install_tile_patch():
    if getattr(bass.Bass, "_wait_split_patched", False):
        return
    orig = bass.Bass.to_json_bytes

    def patched(self):
        return _split_multi_waits(orig(self))

    bass.Bass.to_json_bytes = patched
    bass.Bass._wait_split_patched = True


# --------------------------------------------------------------------------
def _split_blocks(lo, hi, bw_max):
    """Split [lo, hi) into chunks of bw_max with the remainder last."""
    out = []
    off = lo
    while off < hi:
        bw = min(bw_max, hi - off)
        out.append((off, bw))
        off += bw
    return out


class Cfg:
    def __init__(self, H=768, I=2048, A=1965, B=128, BW=512, n_cores=8):
        assert H % P == 0 and I % P == 0
        self.H, self.I, self.A, self.B, self.BW = H, I, A, B, BW
        self.n_cores = n_cores
        self.HC = H // P
        self.IC = I // P
        self.SLOTS = A + B
        self.NSEG = 2 if B > 0 else 1
        # (off, bw, seg): every block lies inside one segment
        self.blocks = [(o, w, 0) for o, w in _split_blocks(0, A, BW)]
        if B > 0:
            self.blocks += [(o, w, 1) for o, w in _split_blocks(A, A + B, BW)]


def build_moe(nc, cfg: Cfg):
    c = cfg
    xT = nc.dram_tensor("xT", [c.HC, P, c.SLOTS], BF16, kind="ExternalInput")
    # w1 is stored mi-major ([NSEG, IC, P, HC*128], host-pretransposed) so one
    # DMA delivers exactly the lhsT columns of one mm1 PSUM group over all kc
    w1T = nc.dram_tensor("w1T", [c.NSEG, c.IC, P, c.HC * P], BF16,
                         kind="ExternalInput")
    w2T = nc.dram_tensor("w2T", [c.NSEG, c.IC, P, c.H], BF16,
                         kind="ExternalInput")
    yT = nc.dram_tensor("yT", [c.HC, P, c.SLOTS], BF16, kind="ExternalOutput")

    with tile_mod.TileContext(nc) as tc:
        _emit(tc, cfg, xT, w1T, w2T, yT)
    return nc


def _emit(tc, c: Cfg, xT, w1T, w2T, yT):
    nc = tc.nc
    ctxs = []

    def pool(**kw):
        p = tc.tile_pool(**kw)
        ctxs.append(p)
        return p.__enter__()

    keep = pool(name="keep", bufs=1)
    hp = pool(name="hp", bufs=2 * c.IC)
    yp = pool(name="yp", bufs=2)
    psum = pool(name="psum", bufs=1, space="PSUM")

    # ---- persistent tiles ----------------------------------------------
    # DMA issue plan (PE consumption order):
    #   Act : first 3 w1[seg0] chunks (Act's sigmoids start later)
    #   SP  : x block 0, w1[seg0] chunks 3..15, x blocks 1.., y writebacks
    #   Pool: w2[seg0], then all segment-1 weights (needed much later)
    xt = [keep.tile([P, c.SLOTS], BF16, name=f"xt{kc}") for kc in range(c.HC)]

    def load_x_block(b):
        boff, bw, _ = c.blocks[b]
        for kc in range(c.HC):
            nc.sync.dma_start(
                out=xt[kc][:, boff : boff + bw], in_=xT[kc][:, boff : boff + bw]
            )

    w1m = [[keep.tile([P, c.HC, P], BF16, name=f"w1m_{s}_{mi}")
            for mi in range(c.IC)] for s in range(c.NSEG)]
    w2t = [[keep.tile([P, c.H], BF16, name=f"w2t_{s}_{k2}")
            for k2 in range(c.IC)] for s in range(c.NSEG)]

    npre = min(3, c.IC)
    for mi in range(npre):
        nc.scalar.dma_start(out=w1m[0][mi][:], in_=w1T[0, mi])
    load_x_block(0)
    for mi in range(npre, c.IC):
        nc.sync.dma_start(out=w1m[0][mi][:], in_=w1T[0, mi])
    for b in range(1, len(c.blocks)):
        load_x_block(b)
    for k2 in range(c.IC):
        nc.gpsimd.dma_start(out=w2t[0][k2][:], in_=w2T[0, k2])
    for s in range(1, c.NSEG):
        for mi in range(c.IC):
            nc.gpsimd.dma_start(out=w1m[s][mi][:], in_=w1T[s, mi])
        for k2 in range(c.IC):
            nc.gpsimd.dma_start(out=w2t[s][k2][:], in_=w2T[s, k2])

    hs = {}

    def mm1(b):
        boff, bw, seg = c.blocks[b]
        for mi in range(c.IC):
            ph = psum.tile([P, c.BW], F32, space="PSUM", name="ph", bufs=3)
            for kc in range(c.HC):
                nc.tensor.matmul(
                    ph[:, :bw],
                    lhsT=w1m[seg][mi][:, kc, :],
                    rhs=xt[kc][:, boff : boff + bw],
                    start=(kc == 0),
                    stop=(kc == c.HC - 1),
                )
            ht = hp.tile([P, c.BW], BF16, name="ht")
            # silu(x) = x * sigmoid(x); Act computes the sigmoid, DVE the mul
            nc.scalar.activation(ht[:, :bw], ph[:, :bw], AF.Sigmoid)
            nc.vector.tensor_mul(ht[:, :bw], ht[:, :bw], ph[:, :bw])
            hs[(b, mi)] = ht

    def mm2(b):
        boff, bw, seg = c.blocks[b]
        yt = yp.tile([P, c.HC, c.BW], BF16, name="yt")
        for hn in range(c.HC):
            py = psum.tile([P, c.BW], F32, space="PSUM", name="py", bufs=3)
            for k2 in range(c.IC):
                nc.tensor.matmul(
                    py[:, :bw],
                    lhsT=w2t[seg][k2][:, hn * P : (hn + 1) * P],
                    rhs=hs[(b, k2)][:, :bw],
                    start=(k2 == 0),
                    stop=(k2 == c.IC - 1),
                )
            # Pool/GPSIMD cannot read PSUM on HW; split the drain copies
            # between DVE and Act
            if hn % 2 == 0:
                nc.vector.tensor_copy(out=yt[:, hn, :bw], in_=py[:, :bw])
            else:
                nc.scalar.activation(yt[:, hn, :bw], py[:, :bw], AF.Copy)
            nc.sync.dma_start(out=yT[hn][:, boff : boff + bw], in_=yt[:, hn, :bw])
        for mi in range(c.IC):
            del hs[(b, mi)]

    # 1-block skew keeps the PE stream dense across the mm1->mm2 boundary
    nb = len(c.blocks)
    mm1(0)
    for b in range(nb):
        if b + 1 < nb:
            mm1(b + 1)
        mm2(b)

    for p in reversed(ctxs):
        p.__exit__(None, None, None)


# --------------------------------------------------------------------------
def route_host(flat, router_w):
    """fp64 router: logits, softmax, top-2, load-balanced gates.

    Returns (gates [T,2] fp64, perm [2T] pair ids sorted stably by expert,
    counts [E])."""
    lg = flat.astype(np.float64) @ router_w.astype(np.float64).T
    order = np.argsort(-lg, axis=1, kind="stable")
    top2 = order[:, :TOP_K]
    mx = lg.max(axis=1, keepdims=True)
    ex = np.exp(lg - mx)
    probs = ex / ex.sum(axis=1, keepdims=True)
    topk_probs = np.take_along_axis(probs, top2, axis=1)
    imp = probs.sum(axis=0)
    running = 1.0 + (1.0 - DECAY) * (imp - 1.0) + EPS
    bal = topk_probs / running[top2]
    gates = bal / bal.sum(axis=1, keepdims=True)
    keys = top2.ravel()
    perm = np.argsort(keys, kind="stable")
    counts = np.bincount(keys, minlength=router_w.shape[0])
    return gates, perm, counts


def _bin_feasible(counts, n, a, b):
    """Can {n bins of a, n bins of b} cover counts?  Returns per-expert
    (p, q) bin usage or None."""
    opts = []
    for cc in counts:
        o = []
        for p_ in range(0, n + 1):
            rem = cc - p_ * a
            q_ = 0 if rem <= 0 else -(-rem // b) if b > 0 else None
            if q_ is not None and q_ <= n:
                o.append((p_, q_))
        if not o:
            return None
        opts.append(o)
    reach = {(0, 0): []}
    for o in opts:
        nxt = {}
        for (sp, sq), path in reach.items():
            for p_, q_ in o:
                k = (sp + p_, sq + q_)
                if k[0] <= n and k[1] <= n and k not in nxt:
                    nxt[k] = path + [(p_, q_)]
        reach = nxt
        if not reach:
            return None
    return next(iter(reach.values()))


def _ok_rem(x, bw):
    r = x % bw
    return r == 0 or r >= P


def solve_layout(counts, n_cores, bw=512):
    """Pick segment sizes (A, B) and per-expert bin usage minimizing
    A+B (per-core slots).  All blocks stay >=128 wide."""
    cmax = int(counts.max())
    # K=1 fallback: one bin per core
    a1 = -(-cmax // 64) * 64
    if not _ok_rem(a1, bw):
        a1 = (a1 // bw) * bw + max(a1 % bw, P)
    best = (a1, 0, [(1, 0)] * len(counts))
    for S in range(int(-(-sum(counts) // n_cores)), a1):
        for b in range(P, S // 2 + 1, 16):
            a = S - b
            if not (_ok_rem(a, bw) and _ok_rem(b, bw)):
                continue
            r = _bin_feasible(counts, n_cores, a, b)
            if r is not None:
                return (a, b, r)
    return best


def assign_bins(counts, usage, n_cores, a, b):
    """Concrete per-core placements.  Returns a list over cores of
    (col_off, bin_cap, expert, pair_off, n_fill)."""
    core_bins = [[] for _ in range(n_cores)]
    free_a = list(range(n_cores))
    free_b = list(range(n_cores))
    for e, (p_, q_) in enumerate(usage):
        left = int(counts[e])
        off = 0
        for _ in range(p_):
            core = free_a.pop(0)
            n_fill = min(left, a)
            core_bins[core].append((0, a, e, off, n_fill))
            left -= n_fill
            off += n_fill
        for _ in range(q_):
            core = free_b.pop(0)
            n_fill = min(left, b)
            core_bins[core].append((a, b, e, off, n_fill))
            left -= n_fill
            off += n_fill
        assert left == 0, (e, counts[e], usage[e])
    return core_bins


def host_prep(flat, router_w, w1, w2, cfg: Cfg, perm, counts, core_bins):
    """Pack per-core xT / per-segment weights from the bin assignment."""
    c = cfg
    bf16 = ml_dtypes.bfloat16
    E = router_w.shape[0]
    tok = perm // TOP_K
    starts = np.concatenate([[0], np.cumsum(counts)])
    # w1 mi-major: W[mi, p, kc*128+j] = w1[e][mi*128+j, kc*128+p]
    w1m = np.ascontiguousarray(
        w1.reshape(E, c.IC, P, c.HC, P).transpose(0, 1, 4, 3, 2)
        .reshape(E, c.IC, P, c.HC * P)
    ).astype(bf16)
    w2T = np.ascontiguousarray(w2.transpose(0, 2, 1)).astype(bf16)  # [E, I, H]
    xbf = flat.astype(bf16)
    in_maps = []
    for core in range(cfg.n_cores):
        xTe = np.zeros((c.H, c.SLOTS), dtype=bf16)
        w1c = np.zeros((c.NSEG, c.IC, P, c.HC * P), dtype=bf16)
        w2c = np.zeros((c.NSEG, c.IC, P, c.H), dtype=bf16)
        for col_off, cap, e, pair_off, n_fill in core_bins[core]:
            sel = tok[starts[e] + pair_off : starts[e] + pair_off + n_fill]
            xTe[:, col_off : col_off + n_fill] = xbf[sel].T
            seg = 0 if col_off == 0 else 1
            w1c[seg] = w1m[e]
            w2c[seg] = w2T[e].reshape(c.IC, P, c.H)
        in_maps.append({"xT": xTe.reshape(c.HC, P, c.SLOTS),
                        "w1T": w1c, "w2T": w2c})
    return in_maps


def host_combine(outs, gates, perm, counts, cfg: Cfg, core_bins):
    """out[t] = sum_k g_k * y[t, e_k] via the bin placement map."""
    c = cfg
    starts = np.concatenate([[0], np.cumsum(counts)])
    T2 = 2 * gates.shape[0]
    y_sorted = np.empty((T2, c.H), dtype=np.float32)
    for core in range(c.n_cores):
        yc = np.asarray(outs[core]).reshape(c.H, c.SLOTS)
        for col_off, cap, e, pair_off, n_fill in core_bins[core]:
            s = starts[e] + pair_off
            y_sorted[s : s + n_fill] = yc[:, col_off : col_off + n_fill].T
    y_pair = np.empty_like(y_sorted)
    y_pair[perm] = y_sorted
    g = gates.astype(np.float32)
    return y_pair[0::2] * g[:, :1] + y_pair[1::2] * g[:, 1:2]


_CACHED = {}


def _get_nc(cfg: Cfg):
    key = (cfg.H, cfg.I, cfg.A, cfg.B, cfg.BW, cfg.n_cores)
    if key not in _CACHED:
        _install_tile_patch()
        nc = bass.Bass("TRN2", num_devices=cfg.n_cores)
        build_moe(nc, cfg)
        _CACHED[key] = nc
    return _CACHED[key]


def run(hidden_states, router_w, w1, w2, cfg: Cfg = None, **run_kwargs):
    from concourse.bass_utils import run_bass_kernel_spmd

    B, S, H = hidden_states.shape
    flat = np.ascontiguousarray(hidden_states.reshape(-1, H).astype(np.float32))
    gates, perm, counts = route_host(flat, router_w)
    n_cores = router_w.shape[0]
    if cfg is None:
        a, b, usage = solve_layout(counts, n_cores)
        cfg = Cfg(H=H, I=w1.shape[1], A=a, B=b, n_cores=n_cores)
    else:
        a, b, usage = solve_layout(counts, n_cores)
        assert (a, b) == (cfg.A, cfg.B), "cfg does not match routing"
    core_bins = assign_bins(counts, usage, n_cores, cfg.A, cfg.B)
    nc = _get_nc(cfg)
    in_maps = host_prep(flat, router_w, w1, w2, cfg, perm, counts, core_bins)
    res = run_bass_kernel_spmd(
        nc, in_maps, core_ids=list(range(cfg.n_cores)), **run_kwargs
    )
    outs = [res.results[i]["yT"] for i in range(cfg.n_cores)]
    full = host_combine(outs, gates, perm, counts, cfg, core_bins)
    return full, res


def kernel(hidden_states, router_w, w1, w2):
    hidden_states = np.asarray(hidden_states, dtype=np.float32)
    router_w = np.asarray(router_w, dtype=np.float32)
    w1 = np.asarray(w1, dtype=np.float32)
    w2 = np.asarray(w2, dtype=np.float32)
    B, S, H = hidden_states.shape
    full, _ = run(hidden_states, router_w, w1, w2)
    return full.reshape(B, S, H).astype(np.float32)
